# revision 8
# baseline (speedup 1.0000x reference)
"""CoGOL ordinal-logistic loss on 8 Trainium2 NeuronCores.

Math (per sample, target t in [1,64], logits x[0..62], x_62 unused):
  masked-logsigmoid sum per row (see reference) equals -Q_i - ln2*[t>=2] with
    Q_i = sum_{j=0}^{t-3} x_j  +  sum_{j=0}^{61} sp(-x_j)  -  sp(-x_{t-2})*[2<=t<=63]
  (sp = softplus). Using sgm = sigmoid(x):  sp(-x_j) = -ln(sgm_j), so
    sum_j sp(-x_j) - sp(-x_{t-2}) = -ln( prod_{j != t-2} sgm_j ).
  result = [sum_i Q_i + ln2*count(t>=2)]/B + a/2*sum(w^2) + b/2*sum(d[1:]^2)

Layout trick: all elementwise tensors are stored TRANSPOSED per tile,
[128, col, row] instead of [128, row, col].  In that orientation the
per-row broadcast of t has its stride-0 dim OUTER and a stride-1 inner
dim, so every DVE op (is_equal one-hot, max-fold, product tree) keeps
the packed-bf16 2x mode (the row-major layout ran them at 1x).

Device (per core, 65536 rows):
  - Act   : sgm_t = sigmoid(x^T) -> bf16 [128, 64, r] (cols 62/63 = 1.0 pad;
            Act engine cost ignores strides so the transposed read is free)
  - DVE   : oh_t = (iota(c+2) == t) bf16 (2x), msel = max(sgm_t, oh_t) (2x)
  - mixed : per-row product via pairwise tree 64->32->16 (2x tensor_tensor,
            per-tile on DVE or gpsimd) or direct full-width Ln on Act --
            per-tile knob to balance engines
  - PE    : C += x_bf16^T @ oh accumulated in PSUM -> [62, 63] class sums
            (x_bf16 = free stride-2 bitcast view = truncated bf16 logits)
  - Act   : Ln over collected [128, 16, 512] partials, accum per partition
Host: prefix term = sum_{j<=c-1} C[j,c]; ln2 count; regularizers; final sum.
"""

import sys

sys.path.insert(0, "/opt/trn_rl_repo")

import numpy as np

ALPHA = 0.01
BETA = 0.05
B = 524288
KM1 = 63
NC62 = 62
NCORES = 8
BC = B // NCORES              # 65536 rows per core
RTOT = BC // 128              # 512 rows per partition total
R = 32                        # max rows per partition per tile
SIZES = [8, 8, 8, 8] + [32] * 15   # small warmup tiles, then full tiles
assert sum(SIZES) == RTOT
TREEW = 16                    # partial products kept per row by the tree
LN2 = 0.6931471805599453

# per-tile engine for the product stage: 'dve'/'gps' = pairwise tree on that
# engine (Ln reads the 16 partials later), 'act' = full-width Ln directly
TREE = ['dve'] * 6 + ['act', 'dve', 'dve', 'act'] + ['dve'] * 9
assert len(TREE) == len(SIZES)
NACT = 2 + sum(1 for e in TREE if e == 'act')   # accum cols: final ln + acts

_PROG = None


def _build():
    import concourse.bacc as bacc
    import concourse.tile as tile
    from concourse import mybir

    f32 = mybir.dt.float32
    bf16 = mybir.dt.bfloat16
    i32 = mybir.dt.int32
    Alu = mybir.AluOpType
    Act = mybir.ActivationFunctionType

    nc = bacc.Bacc("TRN2", target_bir_lowering=False, debug=False,
                   num_devices=NCORES)

    logits = nc.dram_tensor("logits", [BC, KM1], f32, kind="ExternalInput")
    targets = nc.dram_tensor("targets", [BC], bf16, kind="ExternalInput")
    out_ln = nc.dram_tensor("lnacc", [128, NACT], f32, kind="ExternalOutput")
    out_c = nc.dram_tensor("cmat", [NC62, KM1], f32, kind="ExternalOutput")

    with tile.TileContext(nc) as tc:
        with (
            tc.tile_pool(name="const", bufs=1) as cpool,
            tc.tile_pool(name="x", bufs=3) as xpool,
            tc.tile_pool(name="t", bufs=3) as tpool,
            tc.tile_pool(name="s", bufs=3) as spool,
            tc.tile_pool(name="o", bufs=3) as opool,
            tc.tile_pool(name="lns", bufs=2) as lpool,
            tc.tile_pool(name="fin", bufs=1) as fpool,
            tc.tile_pool(name="ps", bufs=1, space="PSUM") as ppool,
        ):
            # iota_rep_t[p, c, r] = c + 2, materialized packed so the
            # transposed one-hot compare keeps all operands stride-1
            iota_i = cpool.tile([128, 64], i32)
            nc.gpsimd.iota(iota_i[:], pattern=[[1, 64]], base=2,
                           channel_multiplier=0)
            iota_f = cpool.tile([128, 64], f32)
            nc.vector.tensor_copy(iota_f[:], iota_i[:])
            iota_b = cpool.tile([128, 64], bf16)
            nc.vector.tensor_copy(iota_b[:], iota_f[:])
            iota_rep = cpool.tile([128, 64, R], bf16)
            nc.vector.tensor_copy(
                iota_rep[:], iota_b[:, :, None].to_broadcast([128, 64, R]))

            # collected partial products, [128, TREEW, RTOT] bf16
            lncol = cpool.tile([128, TREEW, RTOT], bf16)
            acc = cpool.tile([128, NACT], f32)
            psum_c = ppool.tile([NC62, KM1], f32)
            nacti = 2

            roff = 0
            for k, r in enumerate(SIZES):
                rows0 = roff * 128
                xt = xpool.tile([128, R, KM1], f32, tag="x")
                nc.sync.dma_start(
                    xt[:, :r, :],
                    logits.ap()[rows0:rows0 + r * 128, :]
                    .rearrange("(p r) c -> p r c", p=128),
                )
                tt = tpool.tile([128, R], bf16, tag="t")
                nc.sync.dma_start(
                    tt[:, :r],
                    targets.ap()[rows0:rows0 + r * 128]
                    .rearrange("(p r) -> p r", p=128),
                )

                # sgm_t[p, c, r] = sigmoid(x[p, r, c]) -> bf16, 1.0 pad c=62/63
                sgm = spool.tile([128, 64, R], bf16, tag="sgm")
                nc.vector.memset(sgm[:, NC62:64, :r], 1.0)
                nc.scalar.activation(
                    sgm[:, 0:NC62, :r],
                    xt[:, :r, 0:NC62].rearrange("p r c -> p c r"),
                    Act.Sigmoid)

                # one-hot: oh[p, c, r] = (t[p, r] == c+2), 63 cols used
                oh = opool.tile([128, 64, R], bf16, tag="oh")
                nc.vector.tensor_tensor(
                    oh[:, 0:KM1, :r],
                    tt[:, None, :r].to_broadcast([128, KM1, r]),
                    iota_rep[:, 0:KM1, :r],
                    Alu.is_equal,
                )

                # msel = max(sgm, oh) in-place on the 62 real columns (2x)
                nc.vector.tensor_tensor(
                    sgm[:, 0:NC62, :r], sgm[:, 0:NC62, :r],
                    oh[:, 0:NC62, :r], Alu.max)

                # per-row product of msel
                eng = TREE[k]
                if eng == 'act':
                    # this tile skips lncol -- neutralize its slice for the
                    # final Ln pass (gpsimd memset is effectively free)
                    nc.gpsimd.memset(lncol[:, :, roff:roff + r], 1.0)
                    lnsc = lpool.tile([128, NC62, R], f32, tag="lnsc")
                    nc.scalar.activation(
                        lnsc[:, :, :r], sgm[:, 0:NC62, :r], Act.Ln,
                        accum_out=acc[:, nacti:nacti + 1])
                    nacti += 1
                else:
                    e = nc.vector if eng == 'dve' else nc.gpsimd
                    e.tensor_tensor(
                        sgm[:, 0:32, :r], sgm[:, 0:32, :r], sgm[:, 32:64, :r],
                        Alu.mult)
                    e.tensor_tensor(
                        lncol[:, :, roff:roff + r], sgm[:, 0:TREEW, :r],
                        sgm[:, TREEW:32, :r], Alu.mult)

                # PSUM C += x_bf16^T @ oh, contracting the 128 partitions
                xv = xt[:].bitcast(bf16).rearrange(
                    "p r (c two) -> p r c two", two=2)
                for rr in range(r):
                    nc.tensor.matmul(
                        psum_c[:],
                        xv[:, rr, 0:NC62, 1],
                        oh[:, 0:KM1, rr],
                        start=(k == 0 and rr == 0),
                        stop=(k == len(SIZES) - 1 and rr == r - 1),
                    )
                roff += r

            # final: Ln over collected tree partials, accumulate per partition
            ln_out = fpool.tile([128, TREEW, RTOT], f32, tag="lnout")
            nc.scalar.activation(
                ln_out[:], lncol[:], Act.Ln, accum_out=acc[:, 0:1])
            nc.vector.memset(acc[:, 1:2], 0.0)
            nc.sync.dma_start(out_ln.ap(), acc[:])

            cfin = fpool.tile([NC62, KM1], f32, tag="cfin")
            nc.scalar.copy(cfin[:], psum_c[:])
            nc.sync.dma_start(out_c.ap(), cfin[:])

    nc.compile()
    return nc


def _get_prog():
    global _PROG
    if _PROG is None:
        _PROG = _build()
    return _PROG


# host-side prefix weights: C[j, c] counts class t=c+2; row j contributes to
# the prefix sum iff j <= t-3 = c-1
_TRI = (np.arange(NC62)[:, None] <= np.arange(KM1)[None, :] - 1).astype(
    np.float64)


def _in_maps(logits, targets):
    import ml_dtypes

    lg = np.ascontiguousarray(logits, dtype=np.float32)
    tb = np.ascontiguousarray(targets).astype(np.float32).astype(
        ml_dtypes.bfloat16)
    return [
        {
            "logits": lg[c * BC:(c + 1) * BC],
            "targets": tb[c * BC:(c + 1) * BC],
        }
        for c in range(NCORES)
    ]


def kernel(logits, targets, weights, deltas):
    from concourse.bass_utils import run_bass_kernel_spmd

    nc = _get_prog()
    res = run_bass_kernel_spmd(nc, _in_maps(logits, targets),
                               core_ids=list(range(NCORES)))

    total = 0.0
    for c in range(NCORES):
        r = res.results[c]
        total += float((np.asarray(r["cmat"], np.float64) * _TRI).sum())
        total -= float(np.asarray(r["lnacc"], np.float64).sum())

    t64 = np.asarray(targets)
    total += LN2 * float(np.count_nonzero(t64 >= 2))

    w = np.asarray(weights, np.float64)
    d = np.asarray(deltas, np.float64)
    result = (total / B + ALPHA / 2.0 * np.sum(w * w)
              + BETA / 2.0 * np.sum(d[1:] * d[1:]))
    return np.array(result, dtype=np.float32)


# revision 9
# speedup vs baseline: 1.0045x; 1.0045x over previous
"""CoGOL ordinal-logistic loss on 8 Trainium2 NeuronCores.

Math (per sample, target t in [1,64], logits x[0..62], x_62 unused):
  masked-logsigmoid sum per row (see reference) equals -Q_i - ln2*[t>=2] with
    Q_i = sum_{j=0}^{t-3} x_j  +  sum_{j=0}^{61} sp(-x_j)  -  sp(-x_{t-2})*[2<=t<=63]
  (sp = softplus). Using sgm = sigmoid(x):  sp(-x_j) = -ln(sgm_j), so
    sum_j sp(-x_j) - sp(-x_{t-2}) = -ln( prod_{j != t-2} sgm_j ).
  result = [sum_i Q_i + ln2*count(t>=2)]/B + a/2*sum(w^2) + b/2*sum(d[1:]^2)

Layout trick: all elementwise tensors are stored TRANSPOSED per tile,
[128, col, row] instead of [128, row, col].  In that orientation the
per-row broadcast of t has its stride-0 dim OUTER and a stride-1 inner
dim, so every DVE op (is_equal one-hot, max-fold, product tree) keeps
the packed-bf16 2x mode (the row-major layout ran them at 1x).

Device (per core, 65536 rows):
  - Act   : sgm_t = sigmoid(x^T) -> bf16 [128, 64, r] (cols 62/63 = 1.0 pad;
            Act engine cost ignores strides so the transposed read is free)
  - DVE   : oh_t = (iota(c+2) == t) bf16 (2x), msel = max(sgm_t, oh_t) (2x)
  - mixed : per-row product via pairwise tree 64->32->16 (2x tensor_tensor,
            per-tile on DVE or gpsimd) or direct full-width Ln on Act --
            per-tile knob to balance engines
  - PE    : C += x_bf16^T @ oh accumulated in PSUM -> [62, 63] class sums
            (x_bf16 = free stride-2 bitcast view = truncated bf16 logits)
  - Act   : Ln over collected [128, 16, 512] partials, accum per partition
Host: prefix term = sum_{j<=c-1} C[j,c]; ln2 count; regularizers; final sum.
"""

import sys

sys.path.insert(0, "/opt/trn_rl_repo")

import numpy as np

ALPHA = 0.01
BETA = 0.05
B = 524288
KM1 = 63
NC62 = 62
NCORES = 8
BC = B // NCORES              # 65536 rows per core
RTOT = BC // 128              # 512 rows per partition total
R = 32                        # max rows per partition per tile
SIZES = [8, 8, 8, 8] + [32] * 15   # small warmup tiles, then full tiles
assert sum(SIZES) == RTOT
TREEW = 16                    # partial products kept per row by the tree
LN2 = 0.6931471805599453

# per-tile engine for the product stage: 'dve'/'gps' = pairwise tree on that
# engine (Ln reads the 16 partials later), 'act' = full-width Ln directly
TREE = ['dve'] * 6 + ['gps', 'dve', 'dve', 'gps'] + ['dve'] * 9
assert len(TREE) == len(SIZES)
NACT = 2 + sum(1 for e in TREE if e == 'act')   # accum cols: final ln + acts

_PROG = None


def _build():
    import concourse.bacc as bacc
    import concourse.tile as tile
    from concourse import mybir

    f32 = mybir.dt.float32
    bf16 = mybir.dt.bfloat16
    i32 = mybir.dt.int32
    Alu = mybir.AluOpType
    Act = mybir.ActivationFunctionType

    nc = bacc.Bacc("TRN2", target_bir_lowering=False, debug=False,
                   num_devices=NCORES)

    logits = nc.dram_tensor("logits", [BC, KM1], f32, kind="ExternalInput")
    targets = nc.dram_tensor("targets", [BC], bf16, kind="ExternalInput")
    out_ln = nc.dram_tensor("lnacc", [128, NACT], f32, kind="ExternalOutput")
    out_c = nc.dram_tensor("cmat", [NC62, KM1], f32, kind="ExternalOutput")

    with tile.TileContext(nc) as tc:
        with (
            tc.tile_pool(name="const", bufs=1) as cpool,
            tc.tile_pool(name="x", bufs=3) as xpool,
            tc.tile_pool(name="t", bufs=3) as tpool,
            tc.tile_pool(name="s", bufs=3) as spool,
            tc.tile_pool(name="o", bufs=3) as opool,
            tc.tile_pool(name="lns", bufs=2) as lpool,
            tc.tile_pool(name="fin", bufs=1) as fpool,
            tc.tile_pool(name="ps", bufs=1, space="PSUM") as ppool,
        ):
            # iota_rep_t[p, c, r] = c + 2, materialized packed so the
            # transposed one-hot compare keeps all operands stride-1
            iota_i = cpool.tile([128, 64], i32)
            nc.gpsimd.iota(iota_i[:], pattern=[[1, 64]], base=2,
                           channel_multiplier=0)
            iota_f = cpool.tile([128, 64], f32)
            nc.vector.tensor_copy(iota_f[:], iota_i[:])
            iota_b = cpool.tile([128, 64], bf16)
            nc.vector.tensor_copy(iota_b[:], iota_f[:])
            iota_rep = cpool.tile([128, 64, R], bf16)
            nc.vector.tensor_copy(
                iota_rep[:], iota_b[:, :, None].to_broadcast([128, 64, R]))

            # collected partial products, [128, TREEW, RTOT] bf16
            lncol = cpool.tile([128, TREEW, RTOT], bf16)
            acc = cpool.tile([128, NACT], f32)
            psum_c = ppool.tile([NC62, KM1], f32)
            nacti = 2

            roff = 0
            for k, r in enumerate(SIZES):
                rows0 = roff * 128
                xt = xpool.tile([128, R, KM1], f32, tag="x")
                nc.sync.dma_start(
                    xt[:, :r, :],
                    logits.ap()[rows0:rows0 + r * 128, :]
                    .rearrange("(p r) c -> p r c", p=128),
                )
                tt = tpool.tile([128, R], bf16, tag="t")
                nc.sync.dma_start(
                    tt[:, :r],
                    targets.ap()[rows0:rows0 + r * 128]
                    .rearrange("(p r) -> p r", p=128),
                )

                # sgm_t[p, c, r] = sigmoid(x[p, r, c]) -> bf16, 1.0 pad c=62/63
                sgm = spool.tile([128, 64, R], bf16, tag="sgm")
                nc.vector.memset(sgm[:, NC62:64, :r], 1.0)
                nc.scalar.activation(
                    sgm[:, 0:NC62, :r],
                    xt[:, :r, 0:NC62].rearrange("p r c -> p c r"),
                    Act.Sigmoid)

                # one-hot: oh[p, c, r] = (t[p, r] == c+2), 63 cols used
                oh = opool.tile([128, 64, R], bf16, tag="oh")
                nc.vector.tensor_tensor(
                    oh[:, 0:KM1, :r],
                    tt[:, None, :r].to_broadcast([128, KM1, r]),
                    iota_rep[:, 0:KM1, :r],
                    Alu.is_equal,
                )

                # msel = max(sgm, oh) in-place on the 62 real columns (2x)
                nc.vector.tensor_tensor(
                    sgm[:, 0:NC62, :r], sgm[:, 0:NC62, :r],
                    oh[:, 0:NC62, :r], Alu.max)

                # per-row product of msel
                eng = TREE[k]
                if eng == 'act':
                    # this tile skips lncol -- neutralize its slice for the
                    # final Ln pass (gpsimd memset is effectively free)
                    nc.gpsimd.memset(lncol[:, :, roff:roff + r], 1.0)
                    lnsc = lpool.tile([128, NC62, R], f32, tag="lnsc")
                    nc.scalar.activation(
                        lnsc[:, :, :r], sgm[:, 0:NC62, :r], Act.Ln,
                        accum_out=acc[:, nacti:nacti + 1])
                    nacti += 1
                else:
                    e = nc.vector if eng == 'dve' else nc.gpsimd
                    e.tensor_tensor(
                        sgm[:, 0:32, :r], sgm[:, 0:32, :r], sgm[:, 32:64, :r],
                        Alu.mult)
                    e.tensor_tensor(
                        lncol[:, :, roff:roff + r], sgm[:, 0:TREEW, :r],
                        sgm[:, TREEW:32, :r], Alu.mult)

                # PSUM C += x_bf16^T @ oh, contracting the 128 partitions
                xv = xt[:].bitcast(bf16).rearrange(
                    "p r (c two) -> p r c two", two=2)
                for rr in range(r):
                    nc.tensor.matmul(
                        psum_c[:],
                        xv[:, rr, 0:NC62, 1],
                        oh[:, 0:KM1, rr],
                        start=(k == 0 and rr == 0),
                        stop=(k == len(SIZES) - 1 and rr == r - 1),
                    )
                roff += r

            # final: Ln over collected tree partials, accumulate per partition
            ln_out = fpool.tile([128, TREEW, RTOT], f32, tag="lnout")
            nc.scalar.activation(
                ln_out[:], lncol[:], Act.Ln, accum_out=acc[:, 0:1])
            nc.vector.memset(acc[:, 1:2], 0.0)
            nc.sync.dma_start(out_ln.ap(), acc[:])

            cfin = fpool.tile([NC62, KM1], f32, tag="cfin")
            nc.scalar.copy(cfin[:], psum_c[:])
            nc.sync.dma_start(out_c.ap(), cfin[:])

    nc.compile()
    return nc


def _get_prog():
    global _PROG
    if _PROG is None:
        _PROG = _build()
    return _PROG


# host-side prefix weights: C[j, c] counts class t=c+2; row j contributes to
# the prefix sum iff j <= t-3 = c-1
_TRI = (np.arange(NC62)[:, None] <= np.arange(KM1)[None, :] - 1).astype(
    np.float64)


def _in_maps(logits, targets):
    import ml_dtypes

    lg = np.ascontiguousarray(logits, dtype=np.float32)
    tb = np.ascontiguousarray(targets).astype(np.float32).astype(
        ml_dtypes.bfloat16)
    return [
        {
            "logits": lg[c * BC:(c + 1) * BC],
            "targets": tb[c * BC:(c + 1) * BC],
        }
        for c in range(NCORES)
    ]


def kernel(logits, targets, weights, deltas):
    from concourse.bass_utils import run_bass_kernel_spmd

    nc = _get_prog()
    res = run_bass_kernel_spmd(nc, _in_maps(logits, targets),
                               core_ids=list(range(NCORES)))

    total = 0.0
    for c in range(NCORES):
        r = res.results[c]
        total += float((np.asarray(r["cmat"], np.float64) * _TRI).sum())
        total -= float(np.asarray(r["lnacc"], np.float64).sum())

    t64 = np.asarray(targets)
    total += LN2 * float(np.count_nonzero(t64 >= 2))

    w = np.asarray(weights, np.float64)
    d = np.asarray(deltas, np.float64)
    result = (total / B + ALPHA / 2.0 * np.sum(w * w)
              + BETA / 2.0 * np.sum(d[1:] * d[1:]))
    return np.array(result, dtype=np.float32)


# revision 10
# speedup vs baseline: 1.0164x; 1.0119x over previous
"""CoGOL ordinal-logistic loss on 8 Trainium2 NeuronCores.

Math (per sample, target t in [1,64], logits x[0..62], x_62 unused):
  masked-logsigmoid sum per row (see reference) equals -Q_i - ln2*[t>=2] with
    Q_i = sum_{j=0}^{t-3} x_j  +  sum_{j=0}^{61} sp(-x_j)  -  sp(-x_{t-2})*[2<=t<=63]
  (sp = softplus). Using sgm = sigmoid(x):  sp(-x_j) = -ln(sgm_j), so
    sum_j sp(-x_j) - sp(-x_{t-2}) = -ln( prod_{j != t-2} sgm_j ).
  result = [sum_i Q_i + ln2*count(t>=2)]/B + a/2*sum(w^2) + b/2*sum(d[1:]^2)

Layout trick: all elementwise tensors are stored TRANSPOSED per tile,
[128, col, row] instead of [128, row, col].  In that orientation the
per-row broadcast of t has its stride-0 dim OUTER and a stride-1 inner
dim, so every DVE op (is_equal one-hot, max-fold, product tree) keeps
the packed-bf16 2x mode (the row-major layout ran them at 1x).

Device (per core, 65536 rows):
  - Act   : sgm_t = sigmoid(x^T) -> bf16 [128, 64, r] (cols 62/63 = 1.0 pad;
            Act engine cost ignores strides so the transposed read is free)
  - DVE   : oh_t = (iota(c+2) == t) bf16 (2x), msel = max(sgm_t, oh_t) (2x)
  - mixed : per-row product via pairwise tree 64->32->16 (2x tensor_tensor,
            per-tile on DVE or gpsimd) or direct full-width Ln on Act --
            per-tile knob to balance engines
  - PE    : C += x_bf16^T @ oh accumulated in PSUM -> [62, 63] class sums
            (x_bf16 = free stride-2 bitcast view = truncated bf16 logits)
  - Act   : Ln over collected [128, 16, 512] partials, accum per partition
Host: prefix term = sum_{j<=c-1} C[j,c]; ln2 count; regularizers; final sum.
"""

import sys

sys.path.insert(0, "/opt/trn_rl_repo")

import numpy as np

ALPHA = 0.01
BETA = 0.05
B = 524288
KM1 = 63
NC62 = 62
NCORES = 8
BC = B // NCORES              # 65536 rows per core
RTOT = BC // 128              # 512 rows per partition total
R = 32                        # max rows per partition per tile
SIZES = [8, 8, 8, 8] + [32] * 15   # small warmup tiles, then full tiles
assert sum(SIZES) == RTOT
TREEW = 16                    # partial products kept per row by the tree
LN2 = 0.6931471805599453

# per-tile engine for the product stage: 'dve'/'gps' = pairwise tree on that
# engine (Ln reads the 16 partials later), 'act' = full-width Ln directly
TREE = ['dve'] * 19
assert len(TREE) == len(SIZES)
NACT = 2 + sum(1 for e in TREE if e == 'act')   # accum cols: final ln + acts

_PROG = None


def _build():
    import concourse.bacc as bacc
    import concourse.tile as tile
    from concourse import mybir

    f32 = mybir.dt.float32
    bf16 = mybir.dt.bfloat16
    i32 = mybir.dt.int32
    Alu = mybir.AluOpType
    Act = mybir.ActivationFunctionType

    nc = bacc.Bacc("TRN2", target_bir_lowering=False, debug=False,
                   num_devices=NCORES)

    logits = nc.dram_tensor("logits", [BC, KM1], f32, kind="ExternalInput")
    targets = nc.dram_tensor("targets", [BC], bf16, kind="ExternalInput")
    out_ln = nc.dram_tensor("lnacc", [128, NACT], f32, kind="ExternalOutput")
    out_c = nc.dram_tensor("cmat", [NC62, KM1], f32, kind="ExternalOutput")

    with tile.TileContext(nc) as tc:
        with (
            tc.tile_pool(name="const", bufs=1) as cpool,
            tc.tile_pool(name="x", bufs=3) as xpool,
            tc.tile_pool(name="t", bufs=3) as tpool,
            tc.tile_pool(name="s", bufs=3) as spool,
            tc.tile_pool(name="o", bufs=3) as opool,
            tc.tile_pool(name="lns", bufs=2) as lpool,
            tc.tile_pool(name="fin", bufs=1) as fpool,
            tc.tile_pool(name="ps", bufs=1, space="PSUM") as ppool,
        ):
            # iota_rep_t[p, c, r] = c + 2, materialized packed so the
            # transposed one-hot compare keeps all operands stride-1
            iota_i = cpool.tile([128, 64], i32)
            nc.gpsimd.iota(iota_i[:], pattern=[[1, 64]], base=2,
                           channel_multiplier=0)
            iota_f = cpool.tile([128, 64], f32)
            nc.vector.tensor_copy(iota_f[:], iota_i[:])
            iota_b = cpool.tile([128, 64], bf16)
            nc.vector.tensor_copy(iota_b[:], iota_f[:])
            iota_rep = cpool.tile([128, 64, R], bf16)
            nc.vector.tensor_copy(
                iota_rep[:], iota_b[:, :, None].to_broadcast([128, 64, R]))

            # collected partial products, [128, TREEW, RTOT] bf16
            lncol = cpool.tile([128, TREEW, RTOT], bf16)
            acc = cpool.tile([128, NACT], f32)
            psum_c = ppool.tile([NC62, KM1], f32)
            nacti = 2

            roff = 0
            for k, r in enumerate(SIZES):
                rows0 = roff * 128
                xt = xpool.tile([128, R, KM1], f32, tag="x")
                nc.sync.dma_start(
                    xt[:, :r, :],
                    logits.ap()[rows0:rows0 + r * 128, :]
                    .rearrange("(p r) c -> p r c", p=128),
                )
                tt = tpool.tile([128, R], bf16, tag="t")
                nc.sync.dma_start(
                    tt[:, :r],
                    targets.ap()[rows0:rows0 + r * 128]
                    .rearrange("(p r) -> p r", p=128),
                )

                # sgm_t[p, c, r] = sigmoid(x[p, r, c]) -> bf16, 1.0 pad c=62/63
                sgm = spool.tile([128, 64, R], bf16, tag="sgm")
                nc.vector.memset(sgm[:, NC62:64, :r], 1.0)
                nc.scalar.activation(
                    sgm[:, 0:NC62, :r],
                    xt[:, :r, 0:NC62].rearrange("p r c -> p c r"),
                    Act.Sigmoid)

                # one-hot: oh[p, c, r] = (t[p, r] == c+2), 63 cols used
                oh = opool.tile([128, 64, R], bf16, tag="oh")
                nc.vector.tensor_tensor(
                    oh[:, 0:KM1, :r],
                    tt[:, None, :r].to_broadcast([128, KM1, r]),
                    iota_rep[:, 0:KM1, :r],
                    Alu.is_equal,
                )

                # msel = max(sgm, oh) in-place on the 62 real columns (2x)
                nc.vector.tensor_tensor(
                    sgm[:, 0:NC62, :r], sgm[:, 0:NC62, :r],
                    oh[:, 0:NC62, :r], Alu.max)

                # per-row product of msel
                eng = TREE[k]
                if eng == 'act':
                    # this tile skips lncol -- neutralize its slice for the
                    # final Ln pass (gpsimd memset is effectively free)
                    nc.gpsimd.memset(lncol[:, :, roff:roff + r], 1.0)
                    lnsc = lpool.tile([128, NC62, R], f32, tag="lnsc")
                    nc.scalar.activation(
                        lnsc[:, :, :r], sgm[:, 0:NC62, :r], Act.Ln,
                        accum_out=acc[:, nacti:nacti + 1])
                    nacti += 1
                else:
                    e = nc.vector if eng == 'dve' else nc.gpsimd
                    e.tensor_tensor(
                        sgm[:, 0:32, :r], sgm[:, 0:32, :r], sgm[:, 32:64, :r],
                        Alu.mult)
                    e.tensor_tensor(
                        lncol[:, :, roff:roff + r], sgm[:, 0:TREEW, :r],
                        sgm[:, TREEW:32, :r], Alu.mult)

                # PSUM C += x_bf16^T @ oh, contracting the 128 partitions
                xv = xt[:].bitcast(bf16).rearrange(
                    "p r (c two) -> p r c two", two=2)
                for rr in range(r):
                    nc.tensor.matmul(
                        psum_c[:],
                        xv[:, rr, 0:NC62, 1],
                        oh[:, 0:KM1, rr],
                        start=(k == 0 and rr == 0),
                        stop=(k == len(SIZES) - 1 and rr == r - 1),
                    )
                roff += r

            # final: Ln over collected tree partials, accumulate per partition
            ln_out = fpool.tile([128, TREEW, RTOT], f32, tag="lnout")
            nc.scalar.activation(
                ln_out[:], lncol[:], Act.Ln, accum_out=acc[:, 0:1])
            nc.vector.memset(acc[:, 1:2], 0.0)
            nc.sync.dma_start(out_ln.ap(), acc[:])

            cfin = fpool.tile([NC62, KM1], f32, tag="cfin")
            nc.scalar.copy(cfin[:], psum_c[:])
            nc.sync.dma_start(out_c.ap(), cfin[:])

    nc.compile()
    return nc


def _get_prog():
    global _PROG
    if _PROG is None:
        _PROG = _build()
    return _PROG


# host-side prefix weights: C[j, c] counts class t=c+2; row j contributes to
# the prefix sum iff j <= t-3 = c-1
_TRI = (np.arange(NC62)[:, None] <= np.arange(KM1)[None, :] - 1).astype(
    np.float64)


def _in_maps(logits, targets):
    import ml_dtypes

    lg = np.ascontiguousarray(logits, dtype=np.float32)
    tb = np.ascontiguousarray(targets).astype(np.float32).astype(
        ml_dtypes.bfloat16)
    return [
        {
            "logits": lg[c * BC:(c + 1) * BC],
            "targets": tb[c * BC:(c + 1) * BC],
        }
        for c in range(NCORES)
    ]


def kernel(logits, targets, weights, deltas):
    from concourse.bass_utils import run_bass_kernel_spmd

    nc = _get_prog()
    res = run_bass_kernel_spmd(nc, _in_maps(logits, targets),
                               core_ids=list(range(NCORES)))

    total = 0.0
    for c in range(NCORES):
        r = res.results[c]
        total += float((np.asarray(r["cmat"], np.float64) * _TRI).sum())
        total -= float(np.asarray(r["lnacc"], np.float64).sum())

    t64 = np.asarray(targets)
    total += LN2 * float(np.count_nonzero(t64 >= 2))

    w = np.asarray(weights, np.float64)
    d = np.asarray(deltas, np.float64)
    result = (total / B + ALPHA / 2.0 * np.sum(w * w)
              + BETA / 2.0 * np.sum(d[1:] * d[1:]))
    return np.array(result, dtype=np.float32)


# revision 11
# speedup vs baseline: 1.1460x; 1.1275x over previous
"""CoGOL ordinal-logistic loss on 8 Trainium2 NeuronCores.

Math (per sample, target t in [1,64], logits x[0..62], x_62 unused):
  masked-logsigmoid sum per row (see reference) equals -Q_i - ln2*[t>=2] with
    Q_i = sum_{j=0}^{t-3} x_j  +  sum_{j=0}^{61} sp(-x_j)  -  sp(-x_{t-2})*[2<=t<=63]
  (sp = softplus). Using sgm = sigmoid(x):  sp(-x_j) = -ln(sgm_j), so
    sum_j sp(-x_j) - sp(-x_{t-2}) = -ln( prod_{j != t-2} sgm_j ).
  result = [sum_i Q_i + ln2*count(t>=2)]/B + a/2*sum(w^2) + b/2*sum(d[1:]^2)

Layout trick: per-tile tensors use a PAIR-TRANSPOSED layout
[128, r/2, 64, 2] (row pair innermost).  The DVE packed-bf16 2x mode only
requires the innermost AP dim of every operand to be stride-1 and >= 2
elements; with the row-pair innermost, the per-row broadcast of t has its
stride-0 dim in the middle and stays 2x-eligible (row-major ran 1x, and a
full transpose made the Act engine's strided reads 1.7x slower).

Device (per core, 65536 rows):
  - Act   : sgm2 = sigmoid(x) -> bf16 pair layout (sequential reads;
            cols 62/63 padded to 1.0)
  - DVE   : oh2 = (iota(c+2) == t) (2x), msel = max(sgm2, oh2) (2x),
            pairwise product tree 64 -> 32 -> 16 (2x)  [per-tile knob can
            move the product to gpsimd or a full-width Ln on Act]
  - PE    : C += x_bf16^T @ oh accumulated in PSUM -> [62, 63] class sums
            (x_bf16 = free stride-2 bitcast view = truncated bf16 logits)
  - Act   : Ln over collected [128, 16, 512] partials, accum per partition
Host: prefix term = sum_{j<=c-1} C[j,c]; ln2 count; regularizers; final sum.
"""

import sys

sys.path.insert(0, "/opt/trn_rl_repo")

import numpy as np

ALPHA = 0.01
BETA = 0.05
B = 524288
KM1 = 63
NC62 = 62
NCORES = 8
BC = B // NCORES              # 65536 rows per core
RTOT = BC // 128              # 512 rows per partition total
R = 32                        # max rows per partition per tile
SIZES = [8, 8, 8, 8] + [32] * 15   # small warmup tiles, then full tiles
assert sum(SIZES) == RTOT
TREEW = 16                    # partial products kept per row by the tree
LN2 = 0.6931471805599453

# per-tile engine for the product stage: 'dve'/'gps' = pairwise tree on that
# engine (Ln reads the 16 partials later), 'act' = full-width Ln directly
TREE = ['dve'] * 19
assert len(TREE) == len(SIZES)
NACT = 2 + sum(1 for e in TREE if e == 'act')   # accum cols: final ln + acts

_PROG = None


def _build():
    import concourse.bacc as bacc
    import concourse.tile as tile
    from concourse import mybir

    f32 = mybir.dt.float32
    bf16 = mybir.dt.bfloat16
    i32 = mybir.dt.int32
    Alu = mybir.AluOpType
    Act = mybir.ActivationFunctionType

    nc = bacc.Bacc("TRN2", target_bir_lowering=False, debug=False,
                   num_devices=NCORES)

    logits = nc.dram_tensor("logits", [BC, KM1], f32, kind="ExternalInput")
    targets = nc.dram_tensor("targets", [BC], bf16, kind="ExternalInput")
    out_ln = nc.dram_tensor("lnacc", [128, NACT], f32, kind="ExternalOutput")
    out_c = nc.dram_tensor("cmat", [NC62, KM1], f32, kind="ExternalOutput")

    with tile.TileContext(nc) as tc:
        with (
            tc.tile_pool(name="const", bufs=1) as cpool,
            tc.tile_pool(name="x", bufs=3) as xpool,
            tc.tile_pool(name="t", bufs=3) as tpool,
            tc.tile_pool(name="s", bufs=3) as spool,
            tc.tile_pool(name="o", bufs=3) as opool,
            tc.tile_pool(name="lns", bufs=2) as lpool,
            tc.tile_pool(name="fin", bufs=1) as fpool,
            tc.tile_pool(name="ps", bufs=1, space="PSUM") as ppool,
        ):
            # iota2[p, c, q] = c + 2 for the pair layout (q = row parity)
            iota_i = cpool.tile([128, 64], i32)
            nc.gpsimd.iota(iota_i[:], pattern=[[1, 64]], base=2,
                           channel_multiplier=0)
            iota_f = cpool.tile([128, 64], f32)
            nc.vector.tensor_copy(iota_f[:], iota_i[:])
            iota_b = cpool.tile([128, 64], bf16)
            nc.vector.tensor_copy(iota_b[:], iota_f[:])
            iota2 = cpool.tile([128, 64, 2], bf16)
            nc.vector.tensor_copy(
                iota2[:], iota_b[:, :, None].to_broadcast([128, 64, 2]))

            # collected partial products, [128, TREEW, RTOT] bf16
            lncol = cpool.tile([128, TREEW, RTOT], bf16)
            acc = cpool.tile([128, NACT], f32)
            psum_c = ppool.tile([NC62, KM1], f32)
            nacti = 2

            roff = 0
            for k, r in enumerate(SIZES):
                rows0 = roff * 128
                rh = r // 2
                xt = xpool.tile([128, R, KM1], f32, tag="x")
                nc.sync.dma_start(
                    xt[:, :r, :],
                    logits.ap()[rows0:rows0 + r * 128, :]
                    .rearrange("(p r) c -> p r c", p=128),
                )
                tt = tpool.tile([128, R], bf16, tag="t")
                nc.sync.dma_start(
                    tt[:, :r],
                    targets.ap()[rows0:rows0 + r * 128]
                    .rearrange("(p r) -> p r", p=128),
                )

                # sgm2[p, h, c, q] = sigmoid(x[p, 2h+q, c]) -> bf16
                # (x is read sequentially; the pair-layout write is 2-strided)
                sgm = spool.tile([128, R // 2, 64, 2], bf16, tag="sgm")
                nc.vector.memset(sgm[:, :rh, NC62:64, :], 1.0)
                nc.scalar.activation(
                    sgm[:, :rh, 0:NC62, :].rearrange("p h c q -> p h q c"),
                    xt[:, :r, 0:NC62].rearrange("p (h q) c -> p h q c", q=2),
                    Act.Sigmoid)

                # one-hot: oh2[p, h, c, q] = (t[p, 2h+q] == c+2)
                oh = opool.tile([128, R // 2, 64, 2], bf16, tag="oh")
                nc.vector.tensor_tensor(
                    oh[:, :rh, 0:KM1, :],
                    tt[:, :r].rearrange("p (h q) -> p h q", q=2)
                    [:, :, None, :].to_broadcast([128, rh, KM1, 2]),
                    iota2[:, 0:KM1, :][:, None, :, :]
                    .to_broadcast([128, rh, KM1, 2]),
                    Alu.is_equal,
                )

                # msel = max(sgm2, oh2) in-place on the 62 real columns (2x)
                nc.vector.tensor_tensor(
                    sgm[:, :rh, 0:NC62, :], sgm[:, :rh, 0:NC62, :],
                    oh[:, :rh, 0:NC62, :], Alu.max)

                # per-row product of msel
                eng = TREE[k]
                if eng == 'act':
                    # this tile skips lncol -- neutralize its slice for the
                    # final Ln pass (gpsimd memset is effectively free)
                    nc.gpsimd.memset(lncol[:, :, roff:roff + r], 1.0)
                    lnsc = lpool.tile([128, R // 2, NC62, 2], f32, tag="lnsc")
                    nc.scalar.activation(
                        lnsc[:, :rh, :, :], sgm[:, :rh, 0:NC62, :], Act.Ln,
                        accum_out=acc[:, nacti:nacti + 1])
                    nacti += 1
                else:
                    e = nc.vector if eng == 'dve' else nc.gpsimd
                    e.tensor_tensor(
                        sgm[:, :rh, 0:32, :], sgm[:, :rh, 0:32, :],
                        sgm[:, :rh, 32:64, :], Alu.mult)
                    e.tensor_tensor(
                        lncol[:, :, roff:roff + r]
                        .rearrange("p w (h q) -> p h w q", q=2),
                        sgm[:, :rh, 0:TREEW, :],
                        sgm[:, :rh, TREEW:32, :], Alu.mult)

                # PSUM C += x_bf16^T @ oh, contracting the 128 partitions
                xv = xt[:].bitcast(bf16).rearrange(
                    "p r (c two) -> p r c two", two=2)
                for rr in range(r):
                    nc.tensor.matmul(
                        psum_c[:],
                        xv[:, rr, 0:NC62, 1],
                        oh[:, rr // 2, 0:KM1, rr % 2],
                        start=(k == 0 and rr == 0),
                        stop=(k == len(SIZES) - 1 and rr == r - 1),
                    )
                roff += r

            # final: Ln over collected tree partials, accumulate per partition
            ln_out = fpool.tile([128, TREEW, RTOT], f32, tag="lnout")
            nc.scalar.activation(
                ln_out[:], lncol[:], Act.Ln, accum_out=acc[:, 0:1])
            nc.vector.memset(acc[:, 1:2], 0.0)
            nc.sync.dma_start(out_ln.ap(), acc[:])

            cfin = fpool.tile([NC62, KM1], f32, tag="cfin")
            nc.scalar.copy(cfin[:], psum_c[:])
            nc.sync.dma_start(out_c.ap(), cfin[:])

    nc.compile()
    return nc


def _get_prog():
    global _PROG
    if _PROG is None:
        _PROG = _build()
    return _PROG


# host-side prefix weights: C[j, c] counts class t=c+2; row j contributes to
# the prefix sum iff j <= t-3 = c-1
_TRI = (np.arange(NC62)[:, None] <= np.arange(KM1)[None, :] - 1).astype(
    np.float64)


def _in_maps(logits, targets):
    import ml_dtypes

    lg = np.ascontiguousarray(logits, dtype=np.float32)
    tb = np.ascontiguousarray(targets).astype(np.float32).astype(
        ml_dtypes.bfloat16)
    return [
        {
            "logits": lg[c * BC:(c + 1) * BC],
            "targets": tb[c * BC:(c + 1) * BC],
        }
        for c in range(NCORES)
    ]


def kernel(logits, targets, weights, deltas):
    from concourse.bass_utils import run_bass_kernel_spmd

    nc = _get_prog()
    res = run_bass_kernel_spmd(nc, _in_maps(logits, targets),
                               core_ids=list(range(NCORES)))

    total = 0.0
    for c in range(NCORES):
        r = res.results[c]
        total += float((np.asarray(r["cmat"], np.float64) * _TRI).sum())
        total -= float(np.asarray(r["lnacc"], np.float64).sum())

    t64 = np.asarray(targets)
    total += LN2 * float(np.count_nonzero(t64 >= 2))

    w = np.asarray(weights, np.float64)
    d = np.asarray(deltas, np.float64)
    result = (total / B + ALPHA / 2.0 * np.sum(w * w)
              + BETA / 2.0 * np.sum(d[1:] * d[1:]))
    return np.array(result, dtype=np.float32)
